# revision 9
# baseline (speedup 1.0000x reference)
"""Trainium2 Bass kernel for the 2D circulant transform.

Math: per example b,  out[b] = C_s @ inp[b] @ C_h^T  where C_s/C_h are the
circulant matrices of seq_circ (S=4096) and hidden_circ (H=1024).

Implementation notes (v3):
- Data-parallel over batch: core b handles example b (B == 8 cores).
- ALL CRT folds are elementwise on the input and precomputed on host; the
  device receives pre-folded operands (same total bytes as the raw input)
  and does only matmuls + recombines.  Per column group (uH = x_lo + x_hi,
  vH = x_lo - x_hi over the H axis), the shipped [128, 32*512] operand
  block holds chunks [d+re | d+im | d-re | d-im | d2...| u3 | v3]:
  d± are the complex twisted-512 residues of the nega-2048 operand
  (mod x^512 ∓ e^{iπ/4}), d2± the twisted-256 residues of the nega-1024
  operand.  Operands/windows are packed host-side into a handful of wide
  contiguous DMAs spread across engine queues (issue cost ~0.6us each).
- nega-2048 runs as TWO complex twisted-512 products (4 real chains of 8
  matmuls) and nega-1024 as two twisted-256 products (16 matmuls of N=256,
  both branches packed into one PSUM bank) -- half the MACs of the direct
  skew-circulant forms.  nega-512 / cyclic-512 keep the direct form.
- A 128xN tile of any (twisted-)circulant matrix is a sliding window into
  rot[p, f] = w_ext[(f - p) mod 2M]; matrices are never materialized.  CRT
  1/2 factors and twiddles are folded into the host windows.
- fp16 operands/weights, fp32 PSUM.  ScalarE evacuates PSUM to fp16;
  VectorE does recombines on fp16 SBUF at 2x rate; GpSimd takes the
  sqrt(1/2)-scale ops.  Output is written fp16 and upcast on host.
- PSUM tags: epr epi emr emi a0 a1 c3 l3n == 8 banks exactly; stage 2
  reuses them in alternating pairs.
"""
import os
import sys

for _p in ("/opt/trn_rl_repo",):
    if _p not in sys.path and os.path.isdir(_p):
        sys.path.append(_p)

import numpy as np

import concourse.bacc as bacc
import concourse.mybir as mybir
import concourse.tile as tile
from concourse import bass_utils

B, S, H = 8, 4096, 1024
MS, MH = S // 2, H // 2
P = 128
NW = 512
HW2 = 256
F16 = mybir.dt.float16
F32 = mybir.dt.float32
SQ = float(np.sqrt(0.5))

_CACHE = {}

# window layout: (name, width), packed host-side in this order.
WIN_LAYOUT = (
    ("w_tpr", 1408), ("w_tpn", 1408),                      # seg 0 (2816)
    ("w_tpi", 1408), ("w_tmr", 1408), ("w_tmn", 1408), ("w_tmi", 1408),
    ("w2pr", 640), ("w2pi", 640), ("w2pn", 640),           # seg 2 (8448)
    ("w2mr", 640), ("w2mi", 640), ("w2mn", 640),
    ("rot_ccc", 896), ("rot_ccn", 1408),
    ("rot_hc", 896), ("rot_hn", 1408),
)
SEG_SPLIT = (2816, 5632, 8448)   # three const tiles / DMAs
WIN_TOTAL = sum(w for _, w in WIN_LAYOUT)


def _build():
    nc = bacc.Bacc("TRN2", target_bir_lowering=False, debug=False,
                   num_devices=B)
    d_op = [nc.dram_tensor(f"op{g}", [P, 32 * NW], F16,
                           kind="ExternalInput").ap() for g in range(2)]
    d_win = nc.dram_tensor("wins", [P, WIN_TOTAL], F16,
                           kind="ExternalInput").ap()
    out = nc.dram_tensor("out", [S, H], F16, kind="ExternalOutput").ap()

    with tile.TileContext(nc) as tc:
        with tc.tile_pool(name="const", bufs=1) as cpool, \
             tc.tile_pool(name="work", bufs=1) as wpool, \
             tc.tile_pool(name="io", bufs=2) as iopool, \
             tc.tile_pool(name="ps", bufs=1, space="PSUM") as ppool:
            # ---- bulk DMAs: windows (3 segments) + operands (3/group) --
            segs = []
            off = 0
            win = {}
            for w in SEG_SPLIT:
                segs.append((off, cpool.tile([P, w], F16, name=f"wseg{off}")))
                off += w
            off = 0
            for name, w in WIN_LAYOUT:
                for so, st in segs:
                    if so <= off < so + st.shape[1]:
                        win[name] = st[:, off - so:off - so + w]
                        break
                off += w
            nc.sync.dma_start(segs[0][1][:], d_win[:, 0:2816])
            nc.sync.dma_start(segs[2][1][:], d_win[:, 8448:16896])
            nc.gpsimd.dma_start(segs[1][1][:], d_win[:, 2816:8448])

            opd = [None, None]
            opa = [None, None]
            opc = [None, None]
            for g, eng in ((0, nc.scalar), (1, nc.gpsimd)):
                opd[g] = iopool.tile([P, 16 * NW], F16, tag="opd", bufs=2,
                                     name=f"opd_{g}")
                eng.dma_start(opd[g][:, :8 * NW], d_op[g][:, :8 * NW])
                eng.dma_start(opd[g][:, 8 * NW:], d_op[g][:, 8 * NW:16 * NW])
            for g, eng in ((0, nc.scalar), (1, nc.gpsimd)):
                opa[g] = iopool.tile([P, 8 * NW], F16, tag="opa", bufs=1,
                                     name=f"opa_{g}")
                eng.dma_start(opa[g][:], d_op[g][:, 16 * NW:24 * NW])
                opc[g] = iopool.tile([P, 8 * NW], F16, tag="opc", bufs=1,
                                     name=f"opc_{g}")
                eng.dma_start(opc[g][:], d_op[g][:, 24 * NW:32 * NW])

            # y tiles: yy[side][g][kt][spc], side 0 = yp (rows < 2048)
            yy = [[[[None] * 4 for _ in range(4)] for _ in range(2)]
                  for _ in range(2)]

            for mi in range(8):
                g, kt = mi // 4, mi % 4

                def osl(buf, i):
                    """stationary slice: chunk i, columns kt*P..kt*P+P."""
                    c0 = i * NW + kt * P
                    return buf[:, c0:c0 + P]

                def chain(tag, mms, n_w=NW):
                    """mms: list of (psum_lo, buf, chunk, wname, d)."""
                    ps = ppool.tile([P, NW], F32, tag=tag, name=f"p_{tag}_{mi}")
                    n = len(mms)
                    for i, (plo, buf, o, wname, d) in enumerate(mms):
                        nc.tensor.matmul(ps[:, plo:plo + n_w], osl(buf, o),
                                         win[wname][:, d:d + n_w],
                                         start=(i == 0), stop=(i == n - 1))
                    return ps

                def evac(name, ps):
                    t = iopool.tile([P, NW], F16, tag=f"{name}e", bufs=2,
                                    name=f"{name}e_{mi}")
                    nc.scalar.mul(t[:], ps[:], 1.0)
                    return t

                def tt(name, a, b, op_, bufs=1, wid=NW, eng=None):
                    t = iopool.tile([P, wid], F16, tag=name, bufs=bufs,
                                    name=f"{name}_{mi}")
                    eng = eng or nc.vector
                    (eng.tensor_add if op_ == "+" else eng.tensor_sub)(
                        t[:], a, b)
                    return t

                dtw = [(-j * P) % 1024 for j in range(4)]
                od = opd[g]
                p_epr = chain("epr",
                              [(0, od, j, "w_tpr", dtw[j]) for j in range(4)]
                              + [(0, od, 4 + j, "w_tpn", dtw[j]) for j in range(4)])
                p_epi = chain("epi",
                              [(0, od, j, "w_tpi", dtw[j]) for j in range(4)]
                              + [(0, od, 4 + j, "w_tpr", dtw[j]) for j in range(4)])
                e_pr = evac("epr", p_epr)
                e_pi = evac("epi", p_epi)
                p_emr = chain("emr",
                              [(0, od, 8 + j, "w_tmr", dtw[j]) for j in range(4)]
                              + [(0, od, 12 + j, "w_tmn", dtw[j]) for j in range(4)])
                p_emi = chain("emi",
                              [(0, od, 8 + j, "w_tmi", dtw[j]) for j in range(4)]
                              + [(0, od, 12 + j, "w_tmr", dtw[j]) for j in range(4)])
                e_mr = evac("emr", p_emr)
                e_mi = evac("emi", p_emi)

                ne = [None] * 4
                ne[0] = tt("ne0", e_pr[:], e_mr[:], "+")
                ne[2] = tt("ne2", e_pi[:], e_mi[:], "+")
                dre = tt("dre", e_pr[:], e_mr[:], "-")
                dim = tt("dim", e_pi[:], e_mi[:], "-")
                t3 = tt("t3", dre[:], dim[:], "+", eng=nc.gpsimd)
                t4 = tt("t4", dim[:], dre[:], "-", eng=nc.gpsimd)
                ne[1] = iopool.tile([P, NW], F16, tag="ne1", bufs=1,
                                    name=f"ne1_{mi}")
                nc.gpsimd.tensor_scalar_mul(ne[1][:], t3[:], SQ)
                ne[3] = iopool.tile([P, NW], F16, tag="ne3", bufs=1,
                                    name=f"ne3_{mi}")
                nc.gpsimd.tensor_scalar_mul(ne[3][:], t4[:], SQ)

                # nega-1024 as two twisted-256 products, packed per bank
                oa = opa[g]
                d2 = (0, 384)
                p_a0 = chain("a0",
                             [(0, oa, j, "w2pr", d2[j]) for j in range(2)]
                             + [(0, oa, 2 + j, "w2pn", d2[j]) for j in range(2)]
                             + [(HW2, oa, 4 + j, "w2mr", d2[j]) for j in range(2)]
                             + [(HW2, oa, 6 + j, "w2mn", d2[j]) for j in range(2)],
                             n_w=HW2)
                p_a1 = chain("a1",
                             [(0, oa, j, "w2pi", d2[j]) for j in range(2)]
                             + [(0, oa, 2 + j, "w2pr", d2[j]) for j in range(2)]
                             + [(HW2, oa, 4 + j, "w2mi", d2[j]) for j in range(2)]
                             + [(HW2, oa, 6 + j, "w2mr", d2[j]) for j in range(2)],
                             n_w=HW2)
                ar0 = evac("a0", p_a0)
                ar1 = evac("a1", p_a1)
                aa0 = iopool.tile([P, NW], F16, tag="a0e2", bufs=2,
                                  name=f"a0e2_{mi}")
                aa1 = iopool.tile([P, NW], F16, tag="a1e2", bufs=2,
                                  name=f"a1e2_{mi}")
                nc.vector.tensor_add(aa0[:, :HW2], ar0[:, :HW2],
                                     ar0[:, HW2:])
                nc.vector.tensor_add(aa1[:, :HW2], ar1[:, :HW2],
                                     ar1[:, HW2:])
                ddre = tt("ddre", ar0[:, :HW2], ar0[:, HW2:], "-", wid=HW2)
                ddim = tt("ddim", ar1[:, :HW2], ar1[:, HW2:], "-", wid=HW2)
                t3p = tt("t3p", ddre[:], ddim[:], "+", wid=HW2, eng=nc.gpsimd)
                t4p = tt("t4p", ddim[:], ddre[:], "-", wid=HW2, eng=nc.gpsimd)
                nc.gpsimd.tensor_scalar_mul(aa0[:, HW2:], t3p[:], SQ)
                nc.gpsimd.tensor_scalar_mul(aa1[:, HW2:], t4p[:], SQ)

                oc = opc[g]
                p_c3 = chain("c3", [(0, oc, k, "rot_ccc", (-k * P) % 512)
                                    for k in range(4)])
                p_n3 = chain("l3n", [(0, oc, 4 + k, "rot_ccn", (-k * P) % 1024)
                                     for k in range(4)])
                c3e = evac("c3", p_c3)
                n3e = evac("l3n", p_n3)

                e0 = tt("e0", c3e[:], n3e[:], "+")
                e1 = tt("e1", c3e[:], n3e[:], "-")
                yc = [tt("yc0", e0[:], aa0[:], "+"),
                      tt("yc1", e1[:], aa1[:], "+"),
                      tt("yc2", e0[:], aa0[:], "-"),
                      tt("yc3", e1[:], aa1[:], "-")]
                for spc in range(4):
                    yp = wpool.tile([P, NW], F16, name=f"yp_{mi}_{spc}")
                    ym = wpool.tile([P, NW], F16, name=f"ym_{mi}_{spc}")
                    nc.vector.tensor_add(yp[:], yc[spc][:], ne[spc][:])
                    nc.vector.tensor_sub(ym[:], yc[spc][:], ne[spc][:])
                    yy[0][g][kt][spc] = yp
                    yy[1][g][kt][spc] = ym

            # ---- stage 2 ----
            dhc = [(-k * P) % 512 for k in range(4)]
            dhn = [(-k * P) % 1024 for k in range(4)]
            blk = 0
            for spc in range(4):
                for side in range(2):
                    for ss in range(4):
                        ssl = slice(ss * P, (ss + 1) * P)
                        tzc, tzn = ("c3", "l3n") if blk % 2 == 0 else ("a0", "a1")
                        blk += 1
                        zc = ppool.tile([P, NW], F32, tag=tzc,
                                        name=f"zc_{spc}_{side}_{ss}")
                        for kt in range(4):
                            nc.tensor.matmul(zc[:], yy[side][0][kt][spc][:, ssl],
                                             win["rot_hc"][:, dhc[kt]:dhc[kt] + NW],
                                             start=(kt == 0), stop=(kt == 3))
                        zn = ppool.tile([P, NW], F32, tag=tzn,
                                        name=f"zn_{spc}_{side}_{ss}")
                        for kt in range(4):
                            nc.tensor.matmul(zn[:], yy[side][1][kt][spc][:, ssl],
                                             win["rot_hn"][:, dhn[kt]:dhn[kt] + NW],
                                             start=(kt == 0), stop=(kt == 3))
                        zc16 = iopool.tile([P, NW], F16, tag="zc16", bufs=2,
                                           name=f"zc16_{spc}_{side}_{ss}")
                        nc.scalar.mul(zc16[:], zc[:], 1.0)
                        zn16 = iopool.tile([P, NW], F16, tag="zn16", bufs=2,
                                           name=f"zn16_{spc}_{side}_{ss}")
                        nc.scalar.mul(zn16[:], zn[:], 1.0)
                        ob = iopool.tile([P, H], F16, tag="obuf", bufs=3,
                                         name=f"ob_{spc}_{side}_{ss}")
                        nc.vector.tensor_add(ob[:, 0:NW], zc16[:], zn16[:])
                        nc.vector.tensor_sub(ob[:, NW:H], zc16[:], zn16[:])
                        srow = side * MS + spc * NW + ss * P
                        nc.sync.dma_start(out[srow:srow + P, :], ob[:])

    nc.compile()
    return nc


def _rot(vec, width):
    p = np.arange(P)[:, None]
    mod = len(vec)
    return vec[(np.arange(width)[None, :] - p) % mod].astype(np.float16)


def _prep_windows(seq_circ, hidden_circ):
    beta = np.exp(1j * np.pi / 4)
    cs = seq_circ.astype(np.float64)
    cp = 0.5 * (cs[:MS] + cs[MS:])
    cn = 0.5 * (cs[:MS] - cs[MS:])
    cpp = 0.5 * (cp[:1024] + cp[1024:])
    cpn = 0.5 * (cp[:1024] - cp[1024:])
    cppp = 0.5 * (cpp[:512] + cpp[512:])
    cpn3 = 0.5 * (cpp[:512] - cpp[512:])
    bc = cn[:1024] + 1j * cn[1024:]
    bp = 0.5 * (bc[:512] + beta * bc[512:])
    bm = 0.5 * (bc[:512] - beta * bc[512:])
    bext_p = np.concatenate([bp, beta * bp])
    bext_m = np.concatenate([bm, -beta * bm])
    bc2 = cpn[:512] + 1j * cpn[512:]
    b2p = 0.5 * (bc2[:256] + beta * bc2[256:])
    b2m = 0.5 * (bc2[:256] - beta * bc2[256:])
    bext2p = np.concatenate([b2p, beta * b2p])
    bext2m = np.concatenate([b2m, -beta * b2m])
    ch = hidden_circ.astype(np.float64)
    hp = 0.5 * (ch[:MH] + ch[MH:])
    hn = 0.5 * (ch[:MH] - ch[MH:])
    vecs = {
        "w_tpr": bext_p.real, "w_tpn": -bext_p.imag, "w_tpi": bext_p.imag,
        "w_tmr": bext_m.real, "w_tmn": -bext_m.imag, "w_tmi": bext_m.imag,
        "w2pr": bext2p.real, "w2pi": bext2p.imag, "w2pn": -bext2p.imag,
        "w2mr": bext2m.real, "w2mi": bext2m.imag, "w2mn": -bext2m.imag,
        "rot_ccc": cppp, "rot_ccn": np.concatenate([cpn3, -cpn3]),
        "rot_hc": hp, "rot_hn": np.concatenate([hn, -hn]),
    }
    packed = np.concatenate([_rot(vecs[n], w) for n, w in WIN_LAYOUT], axis=1)
    return np.ascontiguousarray(packed)


def _fold_tree(G):
    """G: [B, 4096, C] fp32 -> packed operand chunks [B, 32, 128, C]."""
    u = G[:, :MS] + G[:, MS:]
    v = G[:, :MS] - G[:, MS:]
    u2 = u[:, :1024] + u[:, 1024:]
    v2 = u[:, :1024] - u[:, 1024:]
    u3 = u2[:, :512] + u2[:, 512:]
    v3 = u2[:, :512] - u2[:, 512:]
    cre, cim = v[:, :1024], v[:, 1024:]
    t1 = SQ * (cre[:, 512:] - cim[:, 512:])
    t2 = SQ * (cre[:, 512:] + cim[:, 512:])
    c2re, c2im = v2[:, :512], v2[:, 512:]
    s1 = SQ * (c2re[:, 256:] - c2im[:, 256:])
    s2 = SQ * (c2re[:, 256:] + c2im[:, 256:])
    blocks = np.concatenate([
        cre[:, :512] + t1, cim[:, :512] + t2,      # d+re d+im
        cre[:, :512] - t1, cim[:, :512] - t2,      # d-re d-im
        c2re[:, :256] + s1, c2im[:, :256] + s2,    # d2+re d2+im
        c2re[:, :256] - s1, c2im[:, :256] - s2,    # d2-re d2-im
        u3, v3,
    ], axis=1)
    bb = blocks.shape[0]
    return blocks.reshape(bb, 32, P, blocks.shape[2])


def _prep_ops(input_emb):
    x = np.asarray(input_emb, dtype=np.float32)
    uH = x[:, :, :MH] + x[:, :, MH:]
    vH = x[:, :, :MH] - x[:, :, MH:]
    res = []
    for G in (uH, vH):
        t = _fold_tree(G).astype(np.float16)          # [B, 32, 128, 512]
        t = t.transpose(0, 2, 1, 3).reshape(B, P, 32 * NW)
        res.append(np.ascontiguousarray(t))
    return res


def _run(input_emb, seq_circ, hidden_circ, trace=False):
    if "nc" not in _CACHE:
        _CACHE["nc"] = _build()
    nc = _CACHE["nc"]
    wins = _prep_windows(np.asarray(seq_circ), np.asarray(hidden_circ))
    op0, op1 = _prep_ops(input_emb)
    in_maps = [{"op0": op0[b], "op1": op1[b], "wins": wins}
               for b in range(B)]
    res = bass_utils.run_bass_kernel_spmd(nc, in_maps, core_ids=list(range(B)),
                                          trace=trace)
    outp = np.stack([res.results[b]["out"] for b in range(B)])
    return outp.astype(np.float32), res


def kernel(input_emb, seq_circ, hidden_circ):
    outp, _ = _run(input_emb, seq_circ, hidden_circ, trace=False)
    return outp


# revision 14
# speedup vs baseline: 1.8866x; 1.8866x over previous
"""Trainium2 Bass kernel for the 2D circulant transform.

Math: per example b,  out[b] = C_s @ inp[b] @ C_h^T  where C_s/C_h are the
circulant matrices of seq_circ (S=4096) and hidden_circ (H=1024).

Implementation notes (v3):
- Data-parallel over batch: core b handles example b (B == 8 cores).
- ALL CRT folds are elementwise on the input and precomputed on host; the
  device receives pre-folded operands (same total bytes as the raw input)
  and does only matmuls + recombines.  Per column group (uH = x_lo + x_hi,
  vH = x_lo - x_hi over the H axis), the shipped [128, 32*512] operand
  block holds chunks [d+re | d+im | d-re | d-im | d2...| u3 | v3]:
  d± are the complex twisted-512 residues of the nega-2048 operand
  (mod x^512 ∓ e^{iπ/4}), d2± the twisted-256 residues of the nega-1024
  operand.  Operands/windows are packed host-side into a handful of wide
  contiguous DMAs spread across engine queues (issue cost ~0.6us each).
- nega-2048 runs as TWO complex twisted-512 products (4 real chains of 8
  matmuls) and nega-1024 as two twisted-256 products (16 matmuls of N=256,
  both branches packed into one PSUM bank) -- half the MACs of the direct
  skew-circulant forms.  nega-512 / cyclic-512 keep the direct form.
- A 128xN tile of any (twisted-)circulant matrix is a sliding window into
  rot[p, f] = w_ext[(f - p) mod 2M]; matrices are never materialized.  CRT
  1/2 factors and twiddles are folded into the host windows.
- fp16 operands/weights, fp32 PSUM.  ScalarE evacuates PSUM to fp16;
  VectorE does recombines on fp16 SBUF at 2x rate; GpSimd takes the
  sqrt(1/2)-scale ops.  Output is written fp16 and upcast on host.
- PSUM tags: epr epi emr emi a0 a1 c3 l3n == 8 banks exactly; stage 2
  reuses them in alternating pairs.
"""
import os
import sys

for _p in ("/opt/trn_rl_repo",):
    if _p not in sys.path and os.path.isdir(_p):
        sys.path.append(_p)

import numpy as np

import concourse.bacc as bacc
import concourse.mybir as mybir
import concourse.tile as tile
from concourse import bass_utils

B, S, H = 8, 4096, 1024
MS, MH = S // 2, H // 2
P = 128
NW = 512
HW2 = 256
F16 = mybir.dt.float16
F32 = mybir.dt.float32
SQ = float(np.sqrt(0.5))

_CACHE = {}

# window layout: (name, width), packed host-side in this order.
WIN_LAYOUT = (
    ("w_tpr", 1408), ("w_tpn", 1408),                      # seg 0 (2816)
    ("w_tpi", 1408), ("w_tmr", 1408), ("w_tmn", 1408), ("w_tmi", 1408),
    ("w2pr", 640), ("w2pi", 640), ("w2pn", 640),           # seg 2 (8448)
    ("w2mr", 640), ("w2mi", 640), ("w2mn", 640),
    ("rot_ccc", 896), ("rot_ccn", 1408),
    ("rot_hc", 896), ("rot_hn", 1408),
)
SEG_SPLIT = (2816, 5632, 8448)   # three const tiles / DMAs
WIN_TOTAL = sum(w for _, w in WIN_LAYOUT)


def _build():
    nc = bacc.Bacc("TRN2", target_bir_lowering=False, debug=False,
                   num_devices=B)
    d_op = [nc.dram_tensor(f"op{g}", [P, 32 * NW], F16,
                           kind="ExternalInput").ap() for g in range(2)]
    d_win = nc.dram_tensor("wins", [P, WIN_TOTAL], F16,
                           kind="ExternalInput").ap()
    out = nc.dram_tensor("out", [S, H], F16, kind="ExternalOutput").ap()

    with tile.TileContext(nc) as tc:
        with tc.tile_pool(name="const", bufs=1) as cpool, \
             tc.tile_pool(name="work", bufs=1) as wpool, \
             tc.tile_pool(name="io", bufs=2) as iopool, \
             tc.tile_pool(name="ps", bufs=1, space="PSUM") as ppool:
            # ---- bulk DMAs: windows (3 segments) + operands (3/group) --
            segs = []
            off = 0
            win = {}
            for w in SEG_SPLIT:
                segs.append((off, cpool.tile([P, w], F16, name=f"wseg{off}")))
                off += w
            off = 0
            for name, w in WIN_LAYOUT:
                for so, st in segs:
                    if so <= off < so + st.shape[1]:
                        win[name] = st[:, off - so:off - so + w]
                        break
                off += w
            # all bulk DMAs on the (otherwise idle) sync queue, issue order
            # = consumption order; operand tiles are split per-DMA so tile
            # deps are precise (a chain only waits for the block it reads).
            opq = [[None] * 4 for _ in range(2)]   # d+re d+im d-re d-im
            opa = [None, None]
            opc = [None, None]

            def load_opq(g, q, eng):
                t = iopool.tile([P, 4 * NW], F16, tag=f"opq{q}", bufs=2,
                                name=f"opq{q}_{g}")
                eng.dma_start(t[:], d_op[g][:, q * 4 * NW:(q + 1) * 4 * NW])
                opq[g][q] = t

            def load_ac(g, eng):
                opa[g] = iopool.tile([P, 8 * NW], F16, tag="opa", bufs=1,
                                     name=f"opa_{g}")
                eng.dma_start(opa[g][:], d_op[g][:, 16 * NW:24 * NW])
                opc[g] = iopool.tile([P, 8 * NW], F16, tag="opc", bufs=1,
                                     name=f"opc_{g}")
                eng.dma_start(opc[g][:], d_op[g][:, 24 * NW:32 * NW])

            # three DMA-capable queues, each loaded in first-need order;
            # group-1 opa/opc (whose issue stalls on group-0 chains) go
            # last on sync where nothing queues behind them but stage-2
            # output DMAs.
            nc.sync.dma_start(segs[0][1][:], d_win[:, 0:2816])
            nc.scalar.dma_start(segs[1][1][:], d_win[:, 2816:8448])
            nc.gpsimd.dma_start(segs[2][1][:], d_win[:, 8448:16896])
            load_opq(0, 0, nc.sync)
            load_opq(0, 1, nc.sync)
            load_opq(0, 2, nc.scalar)
            load_opq(0, 3, nc.scalar)
            load_ac(0, nc.gpsimd)
            load_opq(1, 0, nc.gpsimd)
            load_opq(1, 1, nc.gpsimd)
            load_opq(1, 2, nc.gpsimd)
            load_opq(1, 3, nc.gpsimd)
            load_ac(1, nc.sync)

            # y tiles: yy[side][g][kt][spc], side 0 = yp (rows < 2048)
            yy = [[[[None] * 4 for _ in range(4)] for _ in range(2)]
                  for _ in range(2)]

            for mi in range(8):
                g, kt = mi // 4, mi % 4

                def osl(i):
                    """stationary slice: global chunk i, cols kt*P..+P."""
                    if i < 16:
                        buf, j = opq[g][i // 4], i % 4
                    elif i < 24:
                        buf, j = opa[g], i - 16
                    else:
                        buf, j = opc[g], i - 24
                    c0 = j * NW + kt * P
                    return buf[:, c0:c0 + P]

                def chain(tag, mms, n_w=NW):
                    """mms: list of (psum_lo, chunk, wname, d)."""
                    ps = ppool.tile([P, NW], F32, tag=tag, name=f"p_{tag}_{mi}")
                    n = len(mms)
                    for i, (plo, o, wname, d) in enumerate(mms):
                        nc.tensor.matmul(ps[:, plo:plo + n_w], osl(o),
                                         win[wname][:, d:d + n_w],
                                         start=(i == 0), stop=(i == n - 1))
                    return ps

                def evac(name, ps):
                    t = iopool.tile([P, NW], F16, tag=f"{name}e", bufs=2,
                                    name=f"{name}e_{mi}")
                    nc.scalar.mul(t[:], ps[:], 1.0)
                    return t

                def tt(name, a, b, op_, bufs=1, wid=NW, eng=None):
                    t = iopool.tile([P, wid], F16, tag=name, bufs=bufs,
                                    name=f"{name}_{mi}")
                    eng = eng or nc.vector
                    (eng.tensor_add if op_ == "+" else eng.tensor_sub)(
                        t[:], a, b)
                    return t

                dtw = [(-j * P) % 1024 for j in range(4)]
                p_epr = chain("epr",
                              [(0, j, "w_tpr", dtw[j]) for j in range(4)]
                              + [(0, 4 + j, "w_tpn", dtw[j]) for j in range(4)])
                p_epi = chain("epi",
                              [(0, j, "w_tpi", dtw[j]) for j in range(4)]
                              + [(0, 4 + j, "w_tpr", dtw[j]) for j in range(4)])
                e_pr = evac("epr", p_epr)
                e_pi = evac("epi", p_epi)
                p_emr = chain("emr",
                              [(0, 8 + j, "w_tmr", dtw[j]) for j in range(4)]
                              + [(0, 12 + j, "w_tmn", dtw[j]) for j in range(4)])
                p_emi = chain("emi",
                              [(0, 8 + j, "w_tmi", dtw[j]) for j in range(4)]
                              + [(0, 12 + j, "w_tmr", dtw[j]) for j in range(4)])
                e_mr = evac("emr", p_emr)
                e_mi = evac("emi", p_emi)

                ne = [None] * 4
                ne[0] = tt("ne0", e_pr[:], e_mr[:], "+")
                ne[2] = tt("ne2", e_pi[:], e_mi[:], "+")
                dre = tt("dre", e_pr[:], e_mr[:], "-")
                dim = tt("dim", e_pi[:], e_mi[:], "-")
                t3 = tt("t3", dre[:], dim[:], "+")
                t4 = tt("t4", dim[:], dre[:], "-")
                ne[1] = iopool.tile([P, NW], F16, tag="ne1", bufs=1,
                                    name=f"ne1_{mi}")
                nc.vector.tensor_scalar_mul(ne[1][:], t3[:], SQ)
                ne[3] = iopool.tile([P, NW], F16, tag="ne3", bufs=1,
                                    name=f"ne3_{mi}")
                nc.vector.tensor_scalar_mul(ne[3][:], t4[:], SQ)

                # nega-1024 as two twisted-256 products, packed per bank
                d2 = (0, 384)
                p_a0 = chain("a0",
                             [(0, 16 + j, "w2pr", d2[j]) for j in range(2)]
                             + [(0, 18 + j, "w2pn", d2[j]) for j in range(2)]
                             + [(HW2, 20 + j, "w2mr", d2[j]) for j in range(2)]
                             + [(HW2, 22 + j, "w2mn", d2[j]) for j in range(2)],
                             n_w=HW2)
                p_a1 = chain("a1",
                             [(0, 16 + j, "w2pi", d2[j]) for j in range(2)]
                             + [(0, 18 + j, "w2pr", d2[j]) for j in range(2)]
                             + [(HW2, 20 + j, "w2mi", d2[j]) for j in range(2)]
                             + [(HW2, 22 + j, "w2mr", d2[j]) for j in range(2)],
                             n_w=HW2)
                ar0 = evac("a0", p_a0)
                ar1 = evac("a1", p_a1)
                aa0 = iopool.tile([P, NW], F16, tag="a0e2", bufs=2,
                                  name=f"a0e2_{mi}")
                aa1 = iopool.tile([P, NW], F16, tag="a1e2", bufs=2,
                                  name=f"a1e2_{mi}")
                nc.vector.tensor_add(aa0[:, :HW2], ar0[:, :HW2],
                                     ar0[:, HW2:])
                nc.vector.tensor_add(aa1[:, :HW2], ar1[:, :HW2],
                                     ar1[:, HW2:])
                ddre = tt("ddre", ar0[:, :HW2], ar0[:, HW2:], "-", wid=HW2)
                ddim = tt("ddim", ar1[:, :HW2], ar1[:, HW2:], "-", wid=HW2)
                t3p = tt("t3p", ddre[:], ddim[:], "+", wid=HW2)
                t4p = tt("t4p", ddim[:], ddre[:], "-", wid=HW2)
                nc.vector.tensor_scalar_mul(aa0[:, HW2:], t3p[:], SQ)
                nc.vector.tensor_scalar_mul(aa1[:, HW2:], t4p[:], SQ)

                p_c3 = chain("c3", [(0, 24 + k, "rot_ccc", (-k * P) % 512)
                                    for k in range(4)])
                p_n3 = chain("l3n", [(0, 28 + k, "rot_ccn", (-k * P) % 1024)
                                     for k in range(4)])
                c3e = evac("c3", p_c3)
                n3e = evac("l3n", p_n3)

                e0 = tt("e0", c3e[:], n3e[:], "+")
                e1 = tt("e1", c3e[:], n3e[:], "-")
                yc = [tt("yc0", e0[:], aa0[:], "+"),
                      tt("yc1", e1[:], aa1[:], "+"),
                      tt("yc2", e0[:], aa0[:], "-"),
                      tt("yc3", e1[:], aa1[:], "-")]
                for spc in range(4):
                    yp = wpool.tile([P, NW], F16, name=f"yp_{mi}_{spc}")
                    ym = wpool.tile([P, NW], F16, name=f"ym_{mi}_{spc}")
                    nc.vector.tensor_add(yp[:], yc[spc][:], ne[spc][:])
                    nc.gpsimd.tensor_sub(ym[:], yc[spc][:], ne[spc][:])
                    yy[0][g][kt][spc] = yp
                    yy[1][g][kt][spc] = ym

            # ---- stage 2 ----
            dhc = [(-k * P) % 512 for k in range(4)]
            dhn = [(-k * P) % 1024 for k in range(4)]
            blk = 0
            for spc in range(4):
                for side in range(2):
                    for ss in range(4):
                        ssl = slice(ss * P, (ss + 1) * P)
                        tzc, tzn = ("c3", "l3n") if blk % 2 == 0 else ("a0", "a1")
                        blk += 1
                        zc = ppool.tile([P, NW], F32, tag=tzc,
                                        name=f"zc_{spc}_{side}_{ss}")
                        for kt in range(4):
                            nc.tensor.matmul(zc[:], yy[side][0][kt][spc][:, ssl],
                                             win["rot_hc"][:, dhc[kt]:dhc[kt] + NW],
                                             start=(kt == 0), stop=(kt == 3))
                        zn = ppool.tile([P, NW], F32, tag=tzn,
                                        name=f"zn_{spc}_{side}_{ss}")
                        for kt in range(4):
                            nc.tensor.matmul(zn[:], yy[side][1][kt][spc][:, ssl],
                                             win["rot_hn"][:, dhn[kt]:dhn[kt] + NW],
                                             start=(kt == 0), stop=(kt == 3))
                        zc16 = iopool.tile([P, NW], F16, tag="zc16", bufs=2,
                                           name=f"zc16_{spc}_{side}_{ss}")
                        nc.scalar.mul(zc16[:], zc[:], 1.0)
                        zn16 = iopool.tile([P, NW], F16, tag="zn16", bufs=2,
                                           name=f"zn16_{spc}_{side}_{ss}")
                        nc.scalar.mul(zn16[:], zn[:], 1.0)
                        ob = iopool.tile([P, H], F16, tag="obuf", bufs=3,
                                         name=f"ob_{spc}_{side}_{ss}")
                        nc.vector.tensor_add(ob[:, 0:NW], zc16[:], zn16[:])
                        nc.vector.tensor_sub(ob[:, NW:H], zc16[:], zn16[:])
                        srow = side * MS + spc * NW + ss * P
                        nc.sync.dma_start(out[srow:srow + P, :], ob[:])

    nc.compile()
    return nc


def _rot(vec, width):
    p = np.arange(P)[:, None]
    mod = len(vec)
    return vec[(np.arange(width)[None, :] - p) % mod].astype(np.float16)


def _prep_windows(seq_circ, hidden_circ):
    beta = np.exp(1j * np.pi / 4)
    cs = seq_circ.astype(np.float64)
    cp = 0.5 * (cs[:MS] + cs[MS:])
    cn = 0.5 * (cs[:MS] - cs[MS:])
    cpp = 0.5 * (cp[:1024] + cp[1024:])
    cpn = 0.5 * (cp[:1024] - cp[1024:])
    cppp = 0.5 * (cpp[:512] + cpp[512:])
    cpn3 = 0.5 * (cpp[:512] - cpp[512:])
    bc = cn[:1024] + 1j * cn[1024:]
    bp = 0.5 * (bc[:512] + beta * bc[512:])
    bm = 0.5 * (bc[:512] - beta * bc[512:])
    bext_p = np.concatenate([bp, beta * bp])
    bext_m = np.concatenate([bm, -beta * bm])
    bc2 = cpn[:512] + 1j * cpn[512:]
    b2p = 0.5 * (bc2[:256] + beta * bc2[256:])
    b2m = 0.5 * (bc2[:256] - beta * bc2[256:])
    bext2p = np.concatenate([b2p, beta * b2p])
    bext2m = np.concatenate([b2m, -beta * b2m])
    ch = hidden_circ.astype(np.float64)
    hp = 0.5 * (ch[:MH] + ch[MH:])
    hn = 0.5 * (ch[:MH] - ch[MH:])
    vecs = {
        "w_tpr": bext_p.real, "w_tpn": -bext_p.imag, "w_tpi": bext_p.imag,
        "w_tmr": bext_m.real, "w_tmn": -bext_m.imag, "w_tmi": bext_m.imag,
        "w2pr": bext2p.real, "w2pi": bext2p.imag, "w2pn": -bext2p.imag,
        "w2mr": bext2m.real, "w2mi": bext2m.imag, "w2mn": -bext2m.imag,
        "rot_ccc": cppp, "rot_ccn": np.concatenate([cpn3, -cpn3]),
        "rot_hc": hp, "rot_hn": np.concatenate([hn, -hn]),
    }
    packed = np.concatenate([_rot(vecs[n], w) for n, w in WIN_LAYOUT], axis=1)
    return np.ascontiguousarray(packed)


def _fold_tree(G):
    """G: [B, 4096, C] fp32 -> packed operand chunks [B, 32, 128, C]."""
    u = G[:, :MS] + G[:, MS:]
    v = G[:, :MS] - G[:, MS:]
    u2 = u[:, :1024] + u[:, 1024:]
    v2 = u[:, :1024] - u[:, 1024:]
    u3 = u2[:, :512] + u2[:, 512:]
    v3 = u2[:, :512] - u2[:, 512:]
    cre, cim = v[:, :1024], v[:, 1024:]
    t1 = SQ * (cre[:, 512:] - cim[:, 512:])
    t2 = SQ * (cre[:, 512:] + cim[:, 512:])
    c2re, c2im = v2[:, :512], v2[:, 512:]
    s1 = SQ * (c2re[:, 256:] - c2im[:, 256:])
    s2 = SQ * (c2re[:, 256:] + c2im[:, 256:])
    blocks = np.concatenate([
        cre[:, :512] + t1, cim[:, :512] + t2,      # d+re d+im
        cre[:, :512] - t1, cim[:, :512] - t2,      # d-re d-im
        c2re[:, :256] + s1, c2im[:, :256] + s2,    # d2+re d2+im
        c2re[:, :256] - s1, c2im[:, :256] - s2,    # d2-re d2-im
        u3, v3,
    ], axis=1)
    bb = blocks.shape[0]
    return blocks.reshape(bb, 32, P, blocks.shape[2])


def _prep_ops(input_emb):
    x = np.asarray(input_emb, dtype=np.float32)
    uH = x[:, :, :MH] + x[:, :, MH:]
    vH = x[:, :, :MH] - x[:, :, MH:]
    res = []
    for G in (uH, vH):
        t = _fold_tree(G).astype(np.float16)          # [B, 32, 128, 512]
        t = t.transpose(0, 2, 1, 3).reshape(B, P, 32 * NW)
        res.append(np.ascontiguousarray(t))
    return res


def _run(input_emb, seq_circ, hidden_circ, trace=False):
    if "nc" not in _CACHE:
        _CACHE["nc"] = _build()
    nc = _CACHE["nc"]
    wins = _prep_windows(np.asarray(seq_circ), np.asarray(hidden_circ))
    op0, op1 = _prep_ops(input_emb)
    in_maps = [{"op0": op0[b], "op1": op1[b], "wins": wins}
               for b in range(B)]
    res = bass_utils.run_bass_kernel_spmd(nc, in_maps, core_ids=list(range(B)),
                                          trace=trace)
    outp = np.stack([res.results[b]["out"] for b in range(B)])
    return outp.astype(np.float32), res


def kernel(input_emb, seq_circ, hidden_circ):
    outp, _ = _run(input_emb, seq_circ, hidden_circ, trace=False)
    return outp
